# revision 30
# baseline (speedup 1.0000x reference)
"""HSTU-style 4-layer transformer (B=8, T=2048, D=128, H=2) on 8 Trainium2 cores.

Data-parallel over batch: each NeuronCore runs one full sequence.
Residual stream feature-major [D=128 partitions, T=2048 free].

v2 highlights vs v1:
- Single activation table set: FFN gelu computed as silu(1.702x)/1.702
  (1/1.702 folded into c2w) -> no Silu<->Gelu table switches.
- Attention inner loop issues the AV matmul one iteration late (lag-1) so
  the in-order PE queue never blocks behind the Silu->clamp chain.
- Causal triangle applied on DVE via one fused clamp+mask op with a
  two-region AP that skips the dead middle columns; no PE mask-add matmuls.
- Projections batched into [128,1024] activations; V silu writes its
  interleaved [V|ones] layout directly (no gpsimd copy).
- fp32r weights loaded directly via typed DMA (no staging copies).
- ln statistics reduced into one PSUM collector at partitions {0,32,64,96}:
  one DVE copy + one scatter DMA replaces four copies + four DMAs.
- hstu-norm chain batched across all 4 t-chunks (DVE cost is free-size
  bound, so one [128,32] chain covers the whole layer).
"""
import numpy as np
from contextlib import ExitStack

import concourse.bass as bass
import concourse.tile as tile
from concourse import bacc, mybir
from concourse._compat import with_exitstack
from concourse.alu_op_type import AluOpType
from concourse.masks import make_identity

F32 = mybir.dt.float32
F32R = mybir.dt.float32r
BF16 = mybir.dt.bfloat16
I32 = mybir.dt.int32
AF = mybir.ActivationFunctionType
MULT = AluOpType.mult
ADD = AluOpType.add
MAX = AluOpType.max

B, T, D, L, H = 8, 2048, 128, 4, 2
HD = D // H
NITEMS = 200000
EPS = 1e-8
SCALE = 1.0 / np.sqrt(HD)
GELU_A = 1.702
NT = T // 512          # 4 t-chunks of 512
NS = T // 128          # 16 s-chunks of 128
QUAKE_C = 0x5F3759DF


def _quake_rsqrt(eng, pool, v, out_dtype, tag, np_=128):
    """1/sqrt(v) elementwise: quake seed + 2 fused Newton iterations (8 ops)."""
    n = v.shape[-1]
    q1 = pool.tile([np_, n], I32, tag=f"{tag}_q1")
    eng.tensor_scalar(out=q1, in0=v.bitcast(I32), scalar1=1.0,
                      scalar2=None, op0=AluOpType.logical_shift_right)
    q2 = pool.tile([np_, n], I32, tag=f"{tag}_q2")
    eng.tensor_scalar(out=q2, in0=q1, scalar1=-1.0,
                      scalar2=float(QUAKE_C), op0=MULT, op1=ADD)
    cur = q2.bitcast(F32)
    for it in range(2):
        sq = pool.tile([np_, n], F32, tag=f"{tag}_sq{it}")
        eng.tensor_tensor(sq, cur, cur, op=MULT)
        hv = pool.tile([np_, n], F32, tag=f"{tag}_hv{it}")
        eng.scalar_tensor_tensor(out=hv, in0=v, scalar=-0.5,
                                 in1=sq, op0=MULT, op1=MULT)
        nxt = pool.tile([np_, n], out_dtype if it == 1 else F32,
                        tag=f"{tag}_y{it}")
        eng.scalar_tensor_tensor(out=nxt, in0=hv, scalar=1.5,
                                 in1=cur, op0=ADD, op1=MULT)
        cur = nxt
    return cur


def _two_region(t, col0, width):
    """AP over cols [col0, col0+width) and [col0+512, col0+512+width) of a
    [128, 1024]-layout tile (skips the dead middle)."""
    return bass.AP(tensor=t.tensor, offset=t.offset + col0,
                   ap=[t.ap[0], [512, 2], [1, width]])


def _rep_region(t, width):
    """AP repeating cols [0, width) of a [128, >=width] tile twice."""
    return bass.AP(tensor=t.tensor, offset=t.offset,
                   ap=[t.ap[0], [0, 2], [1, width]])


def _stride4(t, off):
    """[64,128] stationary view of sq: col off, off+4, ..., off+508."""
    return bass.AP(tensor=t.tensor, offset=t.offset + off,
                   ap=[t.ap[0], [4, 128]])


@with_exitstack
def _build(ctx: ExitStack, tc: tile.TileContext, io, vb_nonzero: bool):
    nc = tc.nc
    cst = ctx.enter_context(tc.tile_pool(name="cst", bufs=1))
    big = ctx.enter_context(tc.tile_pool(name="big", bufs=1))
    sA = ctx.enter_context(tc.tile_pool(name="sA", bufs=4))
    av = ctx.enter_context(tc.tile_pool(name="av", bufs=2))
    gat = ctx.enter_context(tc.tile_pool(name="gat", bufs=3))
    st = ctx.enter_context(tc.tile_pool(name="st", bufs=2))
    row = ctx.enter_context(tc.tile_pool(name="row", bufs=1))
    stg = ctx.enter_context(tc.tile_pool(name="stg", bufs=2))
    ps_S = ctx.enter_context(tc.tile_pool(name="ps_S", bufs=2, space="PSUM"))
    ps_av = ctx.enter_context(tc.tile_pool(name="ps_av", bufs=1, space="PSUM"))
    ps_b = ctx.enter_context(tc.tile_pool(name="ps_b", bufs=2, space="PSUM"))

    # ---- constants / weights ----
    ident = cst.tile([128, 128], F32)
    make_identity(nc, ident)

    idx = cst.tile([128, NS], I32)
    nc.sync.dma_start(idx, io["idx"])
    posT = cst.tile([128, T], F32)
    nc.scalar.dma_start(posT, io["posT"])

    wr = {}
    _wq = {"wk": nc.sync, "wq": nc.scalar, "wv": nc.gpsimd, "wu": nc.sync,
           "wf2": nc.scalar, "wc1": nc.gpsimd, "wc2": nc.sync}
    for nm in ("wk", "wq", "wv", "wu", "wf2", "wc1", "wc2"):
        f32t = cst.tile([128, L * 128], F32R, tag=f"{nm}_t")
        _wq[nm].dma_start(f32t.rearrange("p (l m) -> p l m", l=L),
                          io[nm].rearrange("l k m -> k l m"))
        wr[nm] = f32t

    sel2 = cst.tile([2, 128], F32R)
    nc.sync.dma_start(sel2, io["sel2"])
    ones1 = cst.tile([1, 128], F32R)
    nc.sync.dma_start(ones1, io["ones1"])
    onesc = cst.tile([128, 1], F32R)
    nc.sync.dma_start(onesc, io["onesc"])
    onesc_b = cst.tile([128, 1], BF16)
    nc.gpsimd.tensor_copy(onesc_b, onesc.bitcast(F32))

    maskT_f = stg.tile([128, 512], F32, tag="maskstg")
    nc.scalar.dma_start(maskT_f, io["maskT"])
    maskT = cst.tile([128, 512], BF16)
    nc.gpsimd.tensor_copy(maskT, maskT_f)

    emb_s = cst.tile([128, 1], F32)
    nc.sync.dma_start(emb_s, io["emb_s"])
    last_s = cst.tile([128, 1], F32)
    nc.sync.dma_start(last_s, io["last_s"])
    bcol = {}
    for nm in ("ub", "qb", "kb", "c1b", "f2b", "c2b"):
        bt = cst.tile([128, L], F32, tag=f"{nm}_t")
        nc.sync.dma_start(bt, io[nm].rearrange("l k -> k l"))
        bcol[nm] = bt
    if vb_nonzero:
        vbB = cst.tile([128, L * 128], F32, tag="vbB")
        nc.sync.dma_start(vbB.rearrange("p (l m) -> p l m", l=L),
                          io["vbB"].rearrange("l p m -> p l m"))

    # v130: per s-chunk [V0(64) | ones | V1(64) | ones]; ones set once.
    v130 = big.tile([128, NS * 130], BF16, tag="v130")
    ones_ap = bass.AP(tensor=v130.tensor, offset=v130.offset + 64,
                      ap=[v130.ap[0], [130, NS], [65, 2], [1, 1]])
    nc.gpsimd.memset(ones_ap, 1.0)

    # ---- helpers ----
    def new_coll():
        """ln-stats collector [128,16]: col 4j+c holds msq of token 512j+128c+p.
        Shares the ps_av slot with avb (lifetimes never overlap)."""
        coll_t = ps_av.tile([128, 16], F32, tag="avb", name="coll_t")
        return coll_t

    def issue_stats(x_t, j, coll):
        """square -> transposed feature-sum into collector cols 4j..4j+3."""
        jc = slice(j * 512, (j + 1) * 512)
        xsq = st.tile([128, 512], BF16, tag="lnxsq")
        nc.gpsimd.tensor_tensor(xsq, x_t[:, jc], x_t[:, jc], op=MULT)
        for c in range(4):
            nc.tensor.matmul(coll[:, 4 * j + c:4 * j + c + 1],
                             xsq[:, 128 * c:128 * (c + 1)], onesc_b,
                             start=True, stop=True)

    def ln_broadcast(coll, tag):
        """collector -> [1,T] rstd row (F32R) via tiny copy/quake/gather."""
        ms_sb = st.tile([128, 16], F32, tag="ms_sb")
        nc.vector.tensor_copy(ms_sb, coll)
        mi = st.tile([128, 16], F32, tag="ln_mi")
        nc.vector.tensor_scalar(out=mi, in0=ms_sb, scalar1=1.0 / D, scalar2=EPS,
                                op0=MULT, op1=ADD)
        rs = _quake_rsqrt(nc, st, mi[:, :], F32, "lnq")
        tp = ps_b.tile([16, 128], F32, tag="pb")
        nc.tensor.transpose(tp, rs, ident)
        rsT = st.tile([16, 128], F32R, tag="rsT")
        nc.vector.tensor_copy(rsT, tp)
        rrow = row.tile([1, T], F32R, tag="lnrow")
        nc.sync.dma_start(rrow, rsT)
        return rrow

    def bcast_row(row_r, j):
        bp = ps_b.tile([128, 512], F32, tag="pb")
        nc.tensor.matmul(bp, ones1, row_r[:, j * 512:(j + 1) * 512],
                         start=True, stop=True)
        return bp

    # ================= embedding gather + transpose + pos =================
    e_sb = big.tile([128, T], F32, tag="e")
    coll = new_coll()
    for g in range(4):
        tr_ps = ps_b.tile([128, 512], F32, tag="pb")
        for c4 in range(4):
            c = 4 * g + c4
            tok = gat.tile([128, 128], F32, tag="tok")
            nc.gpsimd.indirect_dma_start(
                out=tok, out_offset=None, in_=io["itab"][:, :],
                in_offset=bass.IndirectOffsetOnAxis(ap=idx[:, c:c + 1], axis=0))
            nc.tensor.transpose(tr_ps[:, c4 * 128:(c4 + 1) * 128], tok, ident)
        nc.vector.tensor_tensor(e_sb[:, g * 512:(g + 1) * 512], tr_ps,
                                posT[:, g * 512:(g + 1) * 512], op=ADD)
        issue_stats(e_sb, g, coll)

    x_sb = big.tile([128, T], F32, tag="xA")
    er = ln_broadcast(coll, "emb")
    coll = new_coll()
    for j in range(NT):
        jc = slice(j * 512, (j + 1) * 512)
        bp = bcast_row(er, j)
        nc.vector.scalar_tensor_tensor(
            out=x_sb[:, jc], in0=bp, scalar=emb_s[:, 0:1],
            in1=e_sb[:, jc], op0=MULT, op1=MULT)
        issue_stats(x_sb, j, coll)

    # ================= layers =================
    for l in range(L):
        lw = slice(l * 128, (l + 1) * 128)

        # ---- ln1 + U/Q/K/V projections ----
        r1 = ln_broadcast(coll, "ln1")
        xn = big.tile([128, T], F32R, tag="xn")
        for j in range(NT):
            jc = slice(j * 512, (j + 1) * 512)
            bp = bcast_row(r1, j)
            nc.vector.tensor_tensor(xn[:, jc], bp, x_sb[:, jc], op=MULT)

        K = big.tile([128, T], F32R, tag="K")
        Q = big.tile([128, T], F32R, tag="Q")
        U = big.tile([128, T], F32, tag="U")

        def proj_half(wname, bname, dst, h):
            hc = slice(h * 1024, (h + 1) * 1024)
            up = ps_S.tile([128, 1024], F32, tag="S")
            for q in range(2):
                c = slice(h * 1024 + q * 512, h * 1024 + (q + 1) * 512)
                nc.tensor.matmul(up[:, q * 512:(q + 1) * 512], wr[wname][:, lw],
                                 xn[:, c], start=True, stop=True)
            nc.scalar.activation(dst[:, hc], up, AF.Silu,
                                 bias=bcol[bname][:, l:l + 1], scale=1.0)

        def v_group(g):
            vp = ps_b.tile([128, 512], F32, tag="pb")
            for c4 in range(4):
                c = 4 * g + c4
                nc.tensor.matmul(vp[:, c4 * 128:(c4 + 1) * 128],
                                 xn[:, c * 128:(c + 1) * 128], wr["wv"][:, lw],
                                 start=True, stop=True)
            if vb_nonzero:
                vb_ap = bass.AP(tensor=vbB.tensor, offset=vbB.offset + l * 128,
                                ap=[vbB.ap[0], [0, 4], [1, 128]])
                vtmp = st.tile([128, 512], F32, tag="vtmp")
                nc.vector.tensor_tensor(vtmp, vp, vb_ap, op=ADD)
                vsrc = vtmp
            else:
                vsrc = vp
            dst = bass.AP(tensor=v130.tensor, offset=v130.offset + g * 4 * 130,
                          ap=[v130.ap[0], [130, 4], [65, 2], [1, 64]])
            src = bass.AP(tensor=vsrc.tensor, offset=vsrc.offset,
                          ap=[vsrc.ap[0], [128, 4], [64, 2], [1, 64]])
            nc.scalar.activation(dst, src, AF.Silu)

        proj_half("wk", "kb", K, 0)
        proj_half("wq", "qb", Q, 0)
        v_group(0)
        proj_half("wu", "ub", U, 0)

        # ---- attention with lag-1 AV; projections interleaved per j ----
        AVU = big.tile([128, T], F32, tag="AVU")
        pd = row.tile([128, 64], F32, tag="hstu_pd")
        GGrow = row.tile([2, T], F32R, tag="GGrow")
        xsqs = [None] * NT
        x2 = big.tile([128, T], F32, tag="x2")

        def hstu_chain(j0, nj):
            """pd blocks j0..j0+nj-1 -> GGrow segments (per-token G scales)."""
            def blk(off8):
                return bass.AP(tensor=pd.tensor, offset=pd.offset + 16 * j0 + off8,
                               ap=[pd.ap[0], [16, nj], [1, 8]])
            n = 8 * nj
            tg = f"h{nj}"
            de = st.tile([128, n], F32, tag=f"{tg}de")
            nc.vector.tensor_scalar(out=de, in0=blk(0), scalar1=EPS,
                                    scalar2=None, op0=ADD)
            rr = st.tile([128, n], F32, tag=f"{tg}rr")
            scr = st.tile([128, n], F32, tag=f"{tg}scr")
            nc.vector.reciprocal_approx_accurate(rr, de, scratch=scr)
            r2 = st.tile([128, n], F32, tag=f"{tg}r2")
            nc.vector.tensor_tensor(r2, rr, rr, op=MULT)
            uu = st.tile([128, n], F32, tag=f"{tg}uu")
            nc.vector.tensor_tensor(uu, r2, blk(8), op=MULT)

            def h_blk(t, h):
                return bass.AP(tensor=t.tensor, offset=t.offset + 4 * h,
                               ap=[t.ap[0], [8, nj], [1, 4]])
            mm_ = st.tile([128, n // 2], F32, tag=f"{tg}mm")
            nc.vector.tensor_tensor(mm_, h_blk(uu, 0), h_blk(uu, 1), op=ADD)
            mi = st.tile([128, n // 2], F32, tag=f"{tg}mi")
            nc.vector.tensor_scalar(out=mi, in0=mm_, scalar1=1.0 / D,
                                    scalar2=EPS, op0=MULT, op1=ADD)
            Rq = _quake_rsqrt(nc, st, mi[:, :], F32, f"{tg}q")
            GG = st.tile([128, n], F32R, tag=f"{tg}GG")
            nc.vector.tensor_tensor(GG[:, 0:n // 2], h_blk(rr, 0), Rq, op=MULT)
            nc.vector.tensor_tensor(GG[:, n // 2:n], h_blk(rr, 1), Rq, op=MULT)
            for h in range(2):
                for jj in range(nj):
                    nc.gpsimd.dma_start(
                        GGrow[h:h + 1, (j0 + jj) * 512:(j0 + jj + 1) * 512],
                        GG[:, (n // 2) * h + 4 * jj:(n // 2) * h + 4 * jj + 4])

        def f2_one(j):
            jc = slice(j * 512, (j + 1) * 512)
            gb = ps_b.tile([128, 512], F32, tag="pb")
            nc.tensor.matmul(gb, sel2, GGrow[:, jc], start=True, stop=True)
            P = st.tile([128, 512], F32R, tag="Pf2")
            nc.vector.tensor_tensor(P, AVU[:, jc], gb, op=MULT)
            yf = ps_b.tile([128, 512], F32, tag="pb")
            nc.tensor.matmul(yf, wr["wf2"][:, lw], P, start=True, stop=True)
            nc.vector.scalar_tensor_tensor(
                out=x2[:, jc], in0=yf, scalar=bcol["f2b"][:, l:l + 1],
                in1=x_sb[:, jc], op0=ADD, op1=ADD)
            xq = st.tile([128, 512], BF16, tag=f"lnxsq{j}")
            nc.gpsimd.tensor_tensor(xq, x2[:, jc], x2[:, jc], op=MULT)
            xsqs[j] = xq

        for j in range(NT):
            if j == 1:
                v_group(1)
            elif j == 2:
                proj_half("wk", "kb", K, 1)
                proj_half("wq", "qb", Q, 1)
                v_group(2)
                proj_half("wu", "ub", U, 1)
            elif j == 3:
                v_group(3)
                hstu_chain(0, 3)
                for jj in range(3):
                    f2_one(jj)
            jc = slice(j * 512, (j + 1) * 512)
            avb = ps_av.tile([128, 1024], F32, tag="avb")
            nsc = 4 * (j + 1)
            pend = None

            def issue_av(p):
                i, off, A2 = p
                s0 = slice(off, 512)
                s1 = slice(512 + off, 1024)
                nc.tensor.matmul(avb[0:65, s0], v130[:, i * 130:i * 130 + 65],
                                 A2[:, s0], start=(i == 0), stop=(i == nsc - 1))
                nc.tensor.matmul(avb[0:65, s1],
                                 v130[:, i * 130 + 65:i * 130 + 130],
                                 A2[:, s1], start=(i == 0), stop=(i == nsc - 1))

            for i in range(nsc):
                if j == 3 and i == 8:
                    hstu_chain(0, 3, nc.vector, st)
                    for jj in range(3):
                        xq2s.append(f2_one(jj))
                Sp = ps_S.tile([128, 1024], F32, tag="S")
                diag = i >= 4 * j
                off = 128 * (i - 4 * j) if diag else 0
                tq = slice(j * 512 + off, (j + 1) * 512)
                nc.tensor.matmul(Sp[:, off:512], K[0:64, i * 128:(i + 1) * 128],
                                 Q[0:64, tq], start=True, stop=True)
                nc.tensor.matmul(Sp[:, 512 + off:1024],
                                 K[64:128, i * 128:(i + 1) * 128],
                                 Q[64:128, tq], start=True, stop=True)
                A = sA.tile([128, 1024], BF16, tag="A")
                A2 = sA.tile([128, 1024], BF16, tag="A2")
                if diag and off > 0:
                    w = 512 - off
                    nc.scalar.activation(_two_region(A, off, w),
                                         _two_region(Sp, off, w),
                                         AF.Silu, scale=SCALE)
                    nc.vector.scalar_tensor_tensor(
                        out=_two_region(A2, off, w), in0=_two_region(A, off, w),
                        scalar=0.0, in1=_rep_region(maskT, w),
                        op0=MAX, op1=MULT)
                elif diag:
                    nc.scalar.activation(A, Sp, AF.Silu, scale=SCALE)
                    nc.vector.scalar_tensor_tensor(
                        out=A2, in0=A, scalar=0.0,
                        in1=_rep_region(maskT, 512), op0=MAX, op1=MULT)
                else:
                    nc.scalar.activation(A, Sp, AF.Silu, scale=SCALE)
                    nc.vector.tensor_scalar_max(A2, A, 0.0)
                if pend is not None:
                    issue_av(pend)
                pend = (i, off, A2)
            issue_av(pend)

            # j tail: AVU, avb->sbuf, squares, pd scatters (overlaps next j)
            nc.vector.tensor_tensor(AVU[0:64, jc], avb[0:64, 0:512],
                                    U[0:64, jc], op=MULT)
            nc.vector.tensor_tensor(AVU[64:128, jc], avb[0:64, 512:1024],
                                    U[64:128, jc], op=MULT)
            avc = av.tile([128, 1024], F32, tag="avc")
            nc.vector.tensor_copy(avc, avb)
            sq = row.tile([64, 1024], BF16, tag="sq")
            nc.gpsimd.tensor_tensor(sq, avc[0:64, :], avc[0:64, :], op=MULT)
            sscoll = ps_b.tile([128, 8], F32, tag="pb")
            for h in range(2):
                for c in range(4):
                    nc.tensor.matmul(
                        sscoll[:, 4 * h + c:4 * h + c + 1],
                        _stride4(sq, 512 * h + c),
                        onesc_b[0:64, :], start=True, stop=True)
            nc.vector.tensor_copy(pd[:, 16 * j + 8:16 * j + 16], sscoll)
            nc.gpsimd.dma_start(pd[:, 16 * j + 0:16 * j + 4], avc[64:65, 0:512])
            nc.gpsimd.dma_start(pd[:, 16 * j + 4:16 * j + 8], avc[64:65, 512:1024])

        hstu_chain(3, 1)
        f2_one(3)

        # deferred ln2 stat reduces (collector shares the avb psum slot)
        coll = new_coll()
        for j in range(NT):
            for c in range(4):
                nc.tensor.matmul(coll[:, 4 * j + c:4 * j + c + 1],
                                 xsqs[j][:, 128 * c:128 * (c + 1)], onesc_b,
                                 start=True, stop=True)

        # ---- ln2 + FFN (gelu == silu(1.702x)/1.702; 1/1.702 in wc2) ----
        r2row = ln_broadcast(coll, "ln2")
        coll = new_coll()
        xn2 = big.tile([128, T], F32R, tag="xn")
        hh = big.tile([128, T], F32R, tag="U")
        x3 = big.tile([128, T], F32, tag="xB" if l % 2 == 0 else "xA")
        for j in range(NT):
            jc = slice(j * 512, (j + 1) * 512)
            bp = bcast_row(r2row, j)
            nc.vector.tensor_tensor(xn2[:, jc], bp, x2[:, jc], op=MULT)
        for h in range(2):
            hc = slice(h * 1024, (h + 1) * 1024)
            cp = ps_S.tile([128, 1024], F32, tag="S")
            for q in range(2):
                c = slice(h * 1024 + q * 512, h * 1024 + (q + 1) * 512)
                nc.tensor.matmul(cp[:, q * 512:(q + 1) * 512], wr["wc1"][:, lw],
                                 xn2[:, c], start=True, stop=True)
            nc.scalar.activation(hh[:, hc], cp, AF.Silu,
                                 bias=bcol["c1b"][:, l:l + 1], scale=GELU_A)
            for q in range(2):
                j = 2 * h + q
                jcq = slice(j * 512, (j + 1) * 512)
                c2p = ps_b.tile([128, 512], F32, tag="pb")
                nc.tensor.matmul(c2p, wr["wc2"][:, lw], hh[:, jcq],
                                 start=True, stop=True)
                nc.vector.scalar_tensor_tensor(
                    out=x3[:, jcq], in0=c2p, scalar=bcol["c2b"][:, l:l + 1],
                    in1=x2[:, jcq], op0=ADD, op1=ADD)
                issue_stats(x3, j, coll)
        x_sb = x3

    # ================= final norm + output =================
    rf = ln_broadcast(coll, "fin")
    o_sb = big.tile([128, T], F32, tag="e")
    for j in range(NT):
        jc = slice(j * 512, (j + 1) * 512)
        bp = bcast_row(rf, j)
        nc.vector.scalar_tensor_tensor(
            out=o_sb[:, jc], in0=bp, scalar=last_s[:, 0:1],
            in1=x_sb[:, jc], op0=MULT, op1=MULT)
        nc.sync.dma_start(io["out"][:, jc], o_sb[:, jc])


_CACHE = {}


def _get_nc(vb_nonzero: bool):
    key = vb_nonzero
    if key in _CACHE:
        return _CACHE[key]
    nc = bacc.Bacc("TRN2", target_bir_lowering=False, debug=False)
    io = {}
    def din(name, shape, dt=F32):
        io[name] = nc.dram_tensor(name, shape, dt, kind="ExternalInput").ap()
    din("idx", (128, NS), I32)
    din("itab", (NITEMS + 1, 128))
    din("posT", (128, T))
    for nm in ("wq", "wk", "wu", "wv", "wf2", "wc1", "wc2"):
        din(nm, (L, 128, 128), F32R)
    for nm in ("ub", "qb", "kb", "c1b", "f2b", "c2b"):
        din(nm, (L, 128))
    if vb_nonzero:
        din("vbB", (L, 128, 128))
    din("sel2", (2, 128), F32R)
    din("ones1", (1, 128), F32R)
    din("onesc", (128, 1), F32R)
    din("maskT", (128, 512))
    din("emb_s", (128, 1))
    din("last_s", (128, 1))
    io["out"] = nc.dram_tensor("out", (128, T), F32, kind="ExternalOutput").ap()
    with tile.TileContext(nc) as t:
        _build(t, io, vb_nonzero)
    nc.compile()
    _CACHE[key] = nc
    return nc


def _prep_maps(inputs):
    f32 = lambda a: np.ascontiguousarray(np.asarray(a, dtype=np.float32))
    log_seqs = np.asarray(inputs["log_seqs"]).astype(np.int64)
    itab = f32(inputs["item_table"])
    posT = f32(np.asarray(inputs["pos_table"], dtype=np.float32)[1:T + 1].T)
    ln1 = f32(inputs["ln1_s"]); ln2 = f32(inputs["ln2_s"])
    hstu = f32(inputs["hstu_ln_s"])
    com = {
        "itab": itab, "posT": posT,
        "wq": f32(ln1[:, :, None] * np.asarray(inputs["Qw"], np.float32)),
        "wk": f32(ln1[:, :, None] * np.asarray(inputs["Kw"], np.float32)),
        "wu": f32(ln1[:, :, None] * np.asarray(inputs["Uw"], np.float32)),
        "wv": f32(ln1[:, :, None] * np.asarray(inputs["Vw"], np.float32)),
        "wf2": f32(hstu[:, :, None] * np.asarray(inputs["f2w"], np.float32)),
        "wc1": f32(ln2[:, :, None] * np.asarray(inputs["c1w"], np.float32)),
        "wc2": f32(np.asarray(inputs["c2w"], np.float32) / GELU_A),
        "ub": f32(inputs["Ub"]), "qb": f32(inputs["Qb"]), "kb": f32(inputs["Kb"]),
        "c1b": f32(inputs["c1b"]), "f2b": f32(inputs["f2b"]), "c2b": f32(inputs["c2b"]),
        "emb_s": f32(np.asarray(inputs["emb_ln_s"], np.float32).reshape(128, 1)),
        "last_s": f32(np.asarray(inputs["last_ln_s"], np.float32).reshape(128, 1)),
    }
    sel2 = np.zeros((2, 128), np.float32)
    sel2[0, 0:64] = 1.0
    sel2[1, 64:128] = 1.0
    com["sel2"] = sel2
    com["ones1"] = np.ones((1, 128), np.float32)
    com["onesc"] = np.ones((128, 1), np.float32)
    mt = (np.arange(512)[None, :] >= np.arange(128)[:, None]).astype(np.float32)
    com["maskT"] = np.ascontiguousarray(mt)
    vb = np.asarray(inputs["Vb"], np.float32)
    vb_nonzero = bool(np.any(vb != 0.0))
    if vb_nonzero:
        com["vbB"] = f32(np.broadcast_to(vb[:, None, :], (L, 128, 128)))
    maps = []
    for b in range(B):
        m = dict(com)
        m["idx"] = np.ascontiguousarray(
            log_seqs[b].reshape(NS, 128).T.astype(np.int32))
        maps.append(m)
    return maps, vb_nonzero


def kernel(**inputs):
    from concourse.bass_utils import run_bass_kernel_spmd
    maps, vb_nonzero = _prep_maps(inputs)
    nc = _get_nc(vb_nonzero)
    res = run_bass_kernel_spmd(nc, maps, core_ids=list(range(B)))
    out = np.stack([res.results[b]["out"].T for b in range(B)], axis=0)
    return np.ascontiguousarray(out.astype(np.float32))


if __name__ == "__main__":
    # compile-only smoke test
    nc = _get_nc(False)
    import tempfile
    from concourse.bass_utils import compile_bass_kernel
    print("NEFF:", compile_bass_kernel(nc, tempfile.mkdtemp(prefix="hstu_")))
